# revision 1
# baseline (speedup 1.0000x reference)
"""CASSI layer kernel for Trainium2 (8 NeuronCores, Bass/Tile).

Math (matches the reference nn_CASSI_layer):
    H2[m,n,s]        = H[0,m,n,0,s]
    Y[b,m,n+l,s]    += H2[m,n,s] * x[b,m,n,l]            (shear-sum, l in [0,24))
    sigm             = sum(Y^2) / (M*W*B*10^(40/10))
    Yn               = Y + sqrt(sigm) * noise_eps         (noise_eps broadcast over s)
    X[b,m,n,l]       = sum_s H2[m,n,s] * Yn[b,m,n+l,s]
    out              = X / max(X)

Distribution: the (b, m) pairs form 4*256 = 1024 independent rows; each of the
8 cores takes 128 rows (core c: b = c//2, m in [128*(c%2), 128*(c%2)+128)),
mapped onto the 128 SBUF partitions.  Everything per-row lives along the free
dimension, so the spectral shifts are plain address offsets (always 4-byte
aligned in fp16 because the shift stride is S=22 elements).

The two global scalar couplings (sigm, max) are linearized out of the device
kernel: X = X0 + sqrt(sigm)*Xn with X0 the noise-free result (device) and
Xn[b,m,n,l] = (sum_s H2[m,n,s]) * noise_eps[b,m,n+l] (cheap host outer
product).  The device returns X0 and per-partition sum(Y^2); the host applies
sigma, the noise term, and the global max normalization.

Engine split per core: ScalarE materializes the x-column broadcasts over the
s axis; VectorE runs fp16 multiplies/adds in the packed 2x perf mode (the
shear offsets l*S*2 bytes are all 4-byte aligned, and stage-4 pipelines are
pair-batched over l to amortize per-op overhead); GPSIMD owns independent
pipelines for the last few l values in both stages (a second Y accumulator in
stage 2, full mul+fold chains in stage 4); the s-contraction is a
22->16->8->4->2->1 aligned fold tree; and sum(Y^2) rides the ScalarE Square
activation's accumulator.  GPSIMD multiplies read the step-0 broadcast APs
directly (it has no packed perf modes to forfeit), so its chains start right
after the input DMAs with no ScalarE dependency; the first VectorE multiply
likewise runs 1x off the broadcast to skip the ScalarE ramp.  Engine
assignments were tuned with the calibrated instruction-cost timeline
simulator (316us all-VectorE -> 250.6us final; deeper GPSIMD assignment,
cross-engine fold handoffs, emission reorders, strided DMA prefetch/split,
and quad-chunking all measured worse, leaving VectorE's minimal stream --
stage-2 muls+adds, accumulator merge, stage-4 muls+fold trees -- as the
critical path, balanced within ~10us of the GPSIMD chains).
"""

from contextlib import ExitStack

import numpy as np

import concourse.bass as bass
import concourse.bacc as bacc
import concourse.tile as tile
from concourse import mybir
from concourse.bass_utils import run_bass_kernel_spmd

B, M, L, S = 4, 256, 24, 22
W = M + L - 1  # 279
N_CORES = 8
ROWS = 128  # (b, m) rows per core
NOISE_DB = 40.0

_F32 = mybir.dt.float32
_F16 = mybir.dt.float16


def build_bass(dtype=_F16, gps_s2=0, gps_hand_s4=0, tmp_bufs=2, rep_bufs=2, gps_indep_s4=5, gps_indep_s2=5, i2_bufs=2, g2tmp_bufs=1, gpool_bufs=2, s2_stride=100, s4_chunk=4, gps_merge=False, dma_s2=False, s2_chunk=0, s2_chunk_from=1) -> bass.Bass:
    nc = bacc.Bacc()
    x_in = nc.declare_dram_parameter("x_in", [ROWS, M, L], dtype, isOutput=False)
    h_in = nc.declare_dram_parameter("h_in", [ROWS, M, S], dtype, isOutput=False)
    x0_out = nc.declare_dram_parameter("x0_out", [ROWS, M, L], dtype, isOutput=True)
    ss_out = nc.declare_dram_parameter("ss_out", [ROWS, 1], _F32, isOutput=True)

    add = mybir.AluOpType.add

    with tile.TileContext(nc) as tc, ExitStack() as ctx:
        main = ctx.enter_context(tc.tile_pool(name="main", bufs=1))
        reps = ctx.enter_context(tc.tile_pool(name="reps", bufs=rep_bufs))
        tmps = ctx.enter_context(tc.tile_pool(name="tmps", bufs=tmp_bufs))

        xs = main.tile([ROWS, M, L], dtype, tag="xs")
        hs = main.tile([ROWS, M, S], dtype, tag="hs")
        ys = main.tile([ROWS, W, S], dtype, tag="ys")
        ysb = main.tile([ROWS, W, S], dtype, tag="ysb")
        x0 = main.tile([ROWS, M, L], dtype, tag="x0")
        ss = main.tile([ROWS, 1], _F32, tag="ss")

        nc.sync.dma_start(out=hs, in_=h_in[:])
        nc.sync.dma_start(out=xs, in_=x_in[:])
        # ys gets a direct write for l=0 over w in [0, M); only its tail needs
        # zeroing.  ysb (the GPSIMD-side accumulator) is zeroed in full.
        nc.gpsimd.memset(ys[:, M:, :], 0.0)
        if gps_s2 or gps_indep_s2:
            fg = L - gps_s2 - gps_indep_s2
            nc.gpsimd.memset(ysb[:, 0:fg, :], 0.0)
            nc.gpsimd.memset(ysb[:, fg + M :, :], 0.0)

        def x_bcast(l: int) -> bass.AP:
            # x[:, :, l] broadcast along a trailing s axis: [ROWS, M, S]
            sl = xs[:, :, l]
            return bass.AP(
                tensor=sl.tensor, offset=sl.offset, ap=[sl.ap[0], sl.ap[1], [0, S]]
            )

        # Stage 1+2: Y[p, n+l, s] += H[p, n, s] * x[p, n, l]
        # ScalarE materializes the broadcast so VectorE's multiply keeps
        # step-1 fp16 operands (packed 2x mode).  The l-accumulation is split
        # across two buffers so VectorE and GPSIMD own independent chains.
        GPS_S2 = set(range(L - gps_s2, L)) if gps_s2 else set()
        GPS_I2 = set(range(L - gps_s2 - gps_indep_s2, L - gps_s2)) if gps_indep_s2 else set()
        g2tmps = ctx.enter_context(tc.tile_pool(name="g2tmps", bufs=g2tmp_bufs)) if (gps_indep_s2 or gps_s2) else None
        first_gps = min(GPS_S2 | GPS_I2) if (GPS_S2 or GPS_I2) else None
        # interleave GPSIMD l's through the emission order so their ScalarE
        # broadcasts neither starve the VectorE stream nor arrive too late
        gps_ls = sorted(GPS_S2 | GPS_I2)
        dve_s2 = [l for l in range(L) if l not in GPS_S2 and l not in GPS_I2]
        s2_order = []
        gi = 0
        for idx, l in enumerate(dve_s2):
            s2_order.append(l)
            if gi < len(gps_ls) and idx % s2_stride == s2_stride - 1:
                s2_order.append(gps_ls[gi])
                gi += 1
        s2_order.extend(gps_ls[gi:])
        for l in s2_order:
            on_gps = l in GPS_S2 or l in GPS_I2
            if l == 0:
                # direct broadcast read (1x mode) — slower per element but
                # starts as soon as the input DMAs land, before ScalarE's
                # first broadcast copy would be ready
                nc.vector.tensor_mul(out=ys[:, 0:M, :], in0=hs, in1=x_bcast(0))
                continue
            if on_gps:
                # GPSIMD has no packed perf modes, so its multiplies read the
                # step-0 broadcast AP directly — no ScalarE copy needed.
                if l == first_gps:
                    # first GPSIMD l writes ysb directly (no add needed)
                    nc.gpsimd.tensor_mul(
                        out=ysb[:, l : l + M, :], in0=hs, in1=x_bcast(l)
                    )
                else:
                    tmp = g2tmps.tile([ROWS, M, S], dtype, tag="g2tmp")
                    nc.gpsimd.tensor_mul(out=tmp, in0=hs, in1=x_bcast(l))
                    ysl = ysb[:, l : l + M, :]
                    nc.gpsimd.tensor_add(out=ysl, in0=ysl, in1=tmp)
            elif s2_chunk and l >= s2_chunk_from and (l - s2_chunk_from) % 2 == 0 and l + 1 in dve_s2:
                # paired: two ScalarE copies into one double tile, ONE multiply
                xr = reps.tile([ROWS, 2, M, S], dtype, tag="xr")
                nc.scalar.copy(out=xr[:, 0], in_=x_bcast(l))
                nc.scalar.copy(out=xr[:, 1], in_=x_bcast(l + 1))
                tmp = tmps.tile([ROWS, 2, M, S], dtype, tag="tmp")
                nc.vector.tensor_mul(
                    out=tmp,
                    in0=bass.AP(
                        tensor=hs.tensor,
                        offset=hs.offset,
                        ap=[hs.ap[0], [0, 2], [S, M], [1, S]],
                    ),
                    in1=xr,
                )
                for k in range(2):
                    ysl = ys[:, l + k : l + k + M, :]
                    nc.vector.tensor_add(out=ysl, in0=ysl, in1=tmp[:, k])
            elif s2_chunk and l >= s2_chunk_from and (l - s2_chunk_from) % 2 == 1:
                continue  # consumed by the pair above
            else:
                xr = reps.tile([ROWS, M, S], dtype, tag="xr")
                nc.scalar.copy(out=xr, in_=x_bcast(l))
                tmp = tmps.tile([ROWS, M, S], dtype, tag="tmp")
                nc.vector.tensor_mul(out=tmp, in0=hs, in1=xr)
                ysl = ys[:, l : l + M, :]
                nc.vector.tensor_add(out=ysl, in0=ysl, in1=tmp)
        if GPS_S2 or GPS_I2:
            # merge the two accumulators
            merge_eng = nc.gpsimd if gps_merge else nc.vector
            merge_eng.tensor_add(out=ys, in0=ys, in1=ysb)

        # Stage 3 partial: per-partition sum(Y^2) via ScalarE Square+accumulate.
        # ysb is dead after the merge, so it doubles as the Square write target.
        nc.scalar.activation(
            out=ysb, in_=ys, func=mybir.ActivationFunctionType.Square, accum_out=ss
        )
        nc.sync.dma_start(out=ss_out[:], in_=ss)

        # Stage 4: X0[p, n, l] = sum_s H[p, n, s] * Y[p, n+l, s]
        # s-contraction as an aligned fold tree: 22 -> 16 -> 8 -> 4 -> 2 -> 1
        # VectorE does all multiplies; fold chains are split VectorE/GPSIMD.
        FOLDS = ((0, 16, 6), (0, 8, 8), (0, 4, 4), (0, 2, 2))
        GPS_I4 = set(range(L - gps_indep_s4, L)) if gps_indep_s4 else set()
        # handoff l's: VectorE does the multiply, GPSIMD the fold chain
        GPS_H4 = (
            set(range(L - gps_indep_s4 - gps_hand_s4, L - gps_indep_s4))
            if gps_hand_s4
            else set()
        )
        gpool = ctx.enter_context(tc.tile_pool(name="gpool", bufs=gpool_bufs)) if (gps_indep_s4 or gps_indep_s2) else None
        dve_ls = [l for l in range(L) if l not in GPS_I4 and l not in GPS_H4]

        def ap3(t, pair_step, pairs, d1_step, d1_n, d2_step, d2_n, off):
            return bass.AP(
                tensor=t.tensor,
                offset=t.offset + off,
                ap=[t.ap[0], [pair_step, pairs], [d1_step, d1_n], [d2_step, d2_n]],
            )

        # VectorE side: pair-batched pipelines (one mul + one fold tree per
        # two l values, strided across the pair axis of a double-wide tile).
        i = 0
        while i < len(dve_ls):
            l = dve_ls[i]
            npair = 1
            while (
                npair < s4_chunk
                and i + npair < len(dve_ls)
                and dve_ls[i + npair] == l + npair
            ):
                npair += 1
            i += npair
            tmp = tmps.tile([ROWS, npair, M, S], dtype, tag="tmp")
            nc.vector.tensor_mul(
                out=tmp,
                in0=ap3(hs, 0, npair, S, M, 1, S, 0),
                in1=ap3(ys, S, npair, S, M, 1, S, l * S),
            )
            for dst, src, width in FOLDS:
                o = ap3(tmp, M * S, npair, S, M, 1, width, dst)
                nc.vector.tensor_tensor(
                    out=o,
                    in0=o,
                    in1=ap3(tmp, M * S, npair, S, M, 1, width, src),
                    op=add,
                )
            nc.vector.tensor_tensor(
                out=bass.AP(
                    tensor=x0.tensor,
                    offset=x0.offset + l,
                    ap=[x0.ap[0], [1, npair], [L, M]],
                ),
                in0=ap3(tmp, M * S, npair, S, M, 1, 1, 0)[:, :, :, 0],
                in1=ap3(tmp, M * S, npair, S, M, 1, 1, 1)[:, :, :, 0],
                op=add,
            )
        # GPSIMD side: independent single-l pipelines (plus handoff l's whose
        # multiply ran on VectorE).
        for l in sorted(GPS_I4 | GPS_H4):
            tmp = gpool.tile([ROWS, M, S], dtype, tag="gtmp")
            mul_eng = nc.vector if l in GPS_H4 else nc.gpsimd
            mul_eng.tensor_mul(out=tmp, in0=hs, in1=ys[:, l : l + M, :])
            for dst, src, width in FOLDS:
                o = tmp[:, :, dst : dst + width]
                nc.gpsimd.tensor_tensor(
                    out=o, in0=o, in1=tmp[:, :, src : src + width], op=add
                )
            nc.gpsimd.tensor_tensor(
                out=x0[:, :, l], in0=tmp[:, :, 0], in1=tmp[:, :, 1], op=add
            )
        nc.sync.dma_start(out=x0_out[:], in_=x0)

    nc.finalize()
    return nc


def shard_inputs(
    x: np.ndarray, H: np.ndarray, np_dtype=np.float16
) -> list[dict[str, np.ndarray]]:
    H2 = H[0, :, :, 0, :]  # (M, M, S)
    x = x.astype(np_dtype)
    H2 = H2.astype(np_dtype)
    in_maps = []
    for c in range(N_CORES):
        b, half = c // 2, c % 2
        m0 = half * ROWS
        in_maps.append(
            {
                "x_in": np.ascontiguousarray(x[b, m0 : m0 + ROWS]),
                "h_in": np.ascontiguousarray(H2[m0 : m0 + ROWS]),
            }
        )
    return in_maps


def finalize(
    results: list[dict[str, np.ndarray]],
    H: np.ndarray,
    noise_eps: np.ndarray,
) -> np.ndarray:
    X0 = np.empty((B, M, M, L), np.float32)
    sumsq = 0.0
    for c in range(N_CORES):
        b, half = c // 2, c % 2
        m0 = half * ROWS
        X0[b, m0 : m0 + ROWS] = results[c]["x0_out"].astype(np.float32)
        sumsq += results[c]["ss_out"].sum(dtype=np.float64)
    sigm = sumsq / (M * W * B * 10.0 ** (NOISE_DB / 10.0))

    H2 = H[0, :, :, 0, :]  # (M, M, S)
    hsum = H2.sum(axis=-1)  # (M, M)
    # noise window: nwin[b, m, n, l] = noise_eps[b, m, n + l, 0]
    nwin = np.lib.stride_tricks.sliding_window_view(noise_eps[:, :, :, 0], L, axis=2)
    X = X0 + np.float32(np.sqrt(sigm)) * (hsum[None, :, :, None] * nwin)
    X = X.astype(np.float32, copy=False)
    return X / X.max()


_NC_CACHE: bass.Bass | None = None


def kernel(x: np.ndarray, H: np.ndarray, noise_eps: np.ndarray) -> np.ndarray:
    global _NC_CACHE
    x = np.asarray(x, dtype=np.float32)
    H = np.asarray(H, dtype=np.float32)
    noise_eps = np.asarray(noise_eps, dtype=np.float32)
    if _NC_CACHE is None:
        _NC_CACHE = build_bass()
    in_maps = shard_inputs(x, H)
    res = run_bass_kernel_spmd(_NC_CACHE, in_maps, core_ids=list(range(N_CORES)))
    return finalize(res.results, H, noise_eps)



# revision 7
# speedup vs baseline: 3.2964x; 3.2964x over previous
"""CASSI layer kernel for Trainium2 (8 NeuronCores, Bass/Tile) — PE version.

Math (matches the reference nn_CASSI_layer):
    H2[m,n,s]        = H[0,m,n,0,s]
    Y[b,m,n+l,s]    += H2[m,n,s] * x[b,m,n,l]            (shear-sum, l in [0,24))
    sigm             = sum(Y^2) / (M*W*B*10^(40/10))
    Yn               = Y + sqrt(sigm) * noise_eps         (broadcast over s)
    X[b,m,n,l]       = sum_s H2[m,n,s] * Yn[b,m,n+l,s]
    out              = X / max(X)

Distribution: 1024 (b, m) rows over 8 cores (core c: b=c//2, m-half c%2),
128 rows per core.  Both big stages run on the TensorEngine as per-row
matmuls with contraction on SBUF partitions (all operands at partition
base 0 — nonzero PE row tile positions are rejected by the runtime):

  Stage 2 (shear-sum):  Yt_p[s, w] = sum_n H_p[n, s] * B_p[n, w], where
  B_p[n, w] = x_p[n, w-n] is the host-sheared banded matrix (the shear is
  absorbed into the DRAM layout; zero outside the band).  n is chunked by
  64 (chunk c lives on partitions 0-63, slot c), PSUM-accumulating the
  overlapping width-87 w-windows.  Y^T [22(pad 32), 279] fp32 lands
  quad-packed in PSUM (4 rows per 128 partitions via tile_position col
  bases 0/32/64/96) so the fp32->fp16 cast-evacuation runs at full
  128-partition width on DVE/ACT.

  Re-pack: 8 SBUF->SBUF DMAs move the quad-packed Y^T bands down to
  partitions 0-21 (DMA descriptors can cross partitions; compute engines
  cannot), giving yt0[s, g, k, q, w] for stage 4 and the Y output.

  Stage 4 (shots contraction): rect[n', w'] = sum_s H_p[32c+n', s] *
  Y_p[32c+w', s] for n' in [0,32), w' in [0,55).  The needed band
  X0[n, l] = rect[n-32c, n-32c+l] is a diagonal of the rect (crosses
  partitions), so the fp16 rects go to DRAM and the host slices the band.

The two global scalar couplings (sigm, max) are linearized out of the
device program: the device returns noise-free X0 (as rects) plus Y; the
host applies sigma, the noise outer product, and the max normalization.
"""

from contextlib import ExitStack

import numpy as np

import concourse.bass as bass
import concourse.bacc as bacc
import concourse.tile as tile
from concourse import mybir
from concourse.bass_utils import run_bass_kernel_spmd

B, M, L, S = 4, 256, 24, 22
W = M + L - 1  # 279
N_CORES = 8
ROWS = 128
NOISE_DB = 40.0

C2, NCH2, WIN2 = 64, 4, 87  # stage-2 n-chunks
C4, NCH4, WIN4 = 32, 8, 55  # stage-4 n-chunks

_F16 = mybir.dt.float16
_F32 = mybir.dt.float32


def build_bass() -> bass.Bass:
    nc = bacc.Bacc()
    # [n'', c, p, s] = H[p, 64c+n'', s]  (s padded to 32 with zeros)
    hn_in = nc.declare_dram_parameter("hn_in", [64, NCH2, ROWS, 32], _F16, isOutput=False)
    # [n'', c, p, w'] = x[p, 64c+n'', w'-n''] (0 outside band)
    b_in = nc.declare_dram_parameter("b_in", [64, NCH2, ROWS, WIN2], _F16, isOutput=False)
    # [s, g, k, q, n] = H[16g+4q+k, n, s]
    ht_in = nc.declare_dram_parameter("ht_in", [32, 8, 4, 4, M], _F16, isOutput=False)
    # [s, g, k, q, w] = Y[16g+4q+k, w, s]
    yt_out = nc.declare_dram_parameter("yt_out", [S, 8, 4, 4, W], _F16, isOutput=True)
    # [32*(c4%4)+n', w4, c4//4, rw, w'] = rect of row 8*w4+rw, chunk c4
    rect_out = nc.declare_dram_parameter(
        "rect_out", [128, 16, 2, 8, WIN4], _F16, isOutput=True
    )

    with tile.TileContext(nc) as tc, ExitStack() as ctx:
        keep = ctx.enter_context(tc.tile_pool(name="keep", bufs=1))
        s2io = ctx.enter_context(tc.tile_pool(name="s2io", bufs=2))
        psum2 = ctx.enter_context(tc.tile_pool(name="psum2", bufs=2, space="PSUM"))
        psum4 = ctx.enter_context(tc.tile_pool(name="psum4", bufs=2, space="PSUM"))

        ht = keep.tile([32, 8, 4, 4, M], _F16, tag="ht")
        yt16 = keep.tile([128, 8, 4, W], _F16, tag="yt16")
        yt0 = keep.tile([S, 8, 4, 4, W], _F16, tag="yt0")
        for gh in range(2):
            nc.sync.dma_start(out=ht[:, 4 * gh : 4 * gh + 4], in_=ht_in[:, 4 * gh : 4 * gh + 4])
        rpool = ctx.enter_context(tc.tile_pool(name="rpool", bufs=2))

        # ---- stage 2: 16 waves of 8 rows;  p = 16g + 4q + k,
        # wave w2: g = w2//2, j = w2%2, rows (q in 0..4) x (k = 2j + kk) ----
        hts = {}
        bts = {}
        for w2 in range(16):
            g, j = divmod(w2, 2)
            if g not in bts:
                hnt = s2io.tile([64, NCH2, 16, 32], _F16, tag="hns")
                nc.sync.dma_start(out=hnt, in_=hn_in[:, :, 16 * g : 16 * g + 16, :])
                bt = s2io.tile([64, NCH2, 16, WIN2], _F16, tag="bs")
                nc.sync.dma_start(out=bt, in_=b_in[:, :, 16 * g : 16 * g + 16, :])
                hts[g], bts[g] = hnt, bt
            hnt, bt = hts[g], bts[g]
            ps = psum2.tile([128, 2, 512], _F32, tag="ps2")
            for q in range(4):
                for kk in range(2):
                    k = 2 * j + kk
                    p = 16 * g + 4 * q + k
                    pr = p % 16
                    out_sl = ps[32 * q : 32 * q + 32, kk]
                    tp = (0, 32 * q)
                    for c in range(NCH2):
                        lhsT = hnt[:, c, pr, :]
                        rhs = bt[:, c, pr, :]
                        w0 = C2 * c
                        if c == 0:
                            nc.tensor.matmul(
                                out_sl[:, 0:WIN2], lhsT, rhs,
                                start=True, stop=False, tile_position=tp,
                            )
                        else:
                            # close the pending group via the overlap
                            # accumulate, then open one for the fresh columns
                            nc.tensor.matmul(
                                out_sl[:, w0 : w0 + L - 1], lhsT,
                                rhs[:, 0 : L - 1],
                                start=False, stop=True, tile_position=tp,
                            )
                            nc.tensor.matmul(
                                out_sl[:, w0 + L - 1 : w0 + WIN2], lhsT,
                                rhs[:, L - 1 : WIN2],
                                start=True, stop=(c == NCH2 - 1), tile_position=tp,
                            )
            if w2 % 2 == 0:
                nc.vector.tensor_copy(out=yt16[:, g, 2 * j : 2 * j + 2, :], in_=ps[:, :, 0:W])
            else:
                nc.scalar.copy(out=yt16[:, g, 2 * j : 2 * j + 2, :], in_=ps[:, :, 0:W])
            if w2 == 7 or w2 == 15:
                # re-pack the finished g-half down to partitions 0-21
                gh = w2 // 8
                for q in range(4):
                    nc.sync.dma_start(
                        out=yt0[:, 4 * gh : 4 * gh + 4, :, q, :],
                        in_=yt16[32 * q : 32 * q + S, 4 * gh : 4 * gh + 4, :, :],
                    )
                nc.sync.dma_start(
                    out=yt_out[:, 4 * gh : 4 * gh + 4],
                    in_=yt0[:, 4 * gh : 4 * gh + 4],
                )

        # ---- stage 4: 16 waves of 8 rows; p = 8*w4 + rw ----
        for w4 in range(16):
            ps = psum4.tile([128, 2, 512], _F32, tag="ps4")
            for rw in range(8):
                p = 8 * w4 + rw
                g, r = divmod(p, 16)
                q, k = divmod(r, 4)
                for c4 in range(NCH4):
                    cb, bk = c4 % 4, c4 // 4
                    lhsT = ht[0:S, g, k, q, 32 * c4 : 32 * c4 + C4]
                    rhs = yt0[:, g, k, q, 32 * c4 : 32 * c4 + WIN4]
                    nc.tensor.matmul(
                        ps[32 * cb : 32 * cb + C4, bk, 55 * rw : 55 * rw + WIN4],
                        lhsT, rhs,
                        start=True, stop=True, tile_position=(0, 32 * cb),
                    )
            if w4 % 4 == 0:
                rcur = rpool.tile([128, 4, 2, 8, WIN4], _F16, tag="rect")
            dst = rcur[:, w4 % 4].rearrange("p b r w -> p b (r w)")
            if w4 % 2 == 0:
                nc.vector.tensor_copy(out=dst, in_=ps[:, :, 0 : 55 * 8])
            else:
                nc.scalar.copy(out=dst, in_=ps[:, :, 0 : 55 * 8])
            if w4 % 4 == 3:
                rg = w4 // 4
                nc.sync.dma_start(out=rect_out[:, 4 * rg : 4 * rg + 4], in_=rcur)
    nc.finalize()
    return nc


def shard_inputs(x: np.ndarray, H: np.ndarray) -> list[dict[str, np.ndarray]]:
    H2 = H[0, :, :, 0, :].astype(np.float16)  # (M, M, S)
    x16 = x.astype(np.float16)
    nn64 = np.arange(64)
    in_maps = []
    for core in range(N_CORES):
        b, half = core // 2, core % 2
        m0 = half * ROWS
        xr = x16[b, m0 : m0 + ROWS]  # [p, n, l]
        hr = H2[m0 : m0 + ROWS]  # [p, n, s]

        hr4 = hr.reshape(ROWS, 4, 64, S)  # [p, c, n'', s]
        hn = np.zeros((64, NCH2, ROWS, 32), np.float16)
        for c in range(NCH2):
            hn[:, c, :, :S] = hr4[:, c].transpose(1, 0, 2)

        bsh = np.zeros((64, NCH2, ROWS, WIN2), np.float16)
        for c in range(NCH2):
            for d in range(L):
                bsh[nn64, c, :, nn64 + d] = xr[:, 64 * c + nn64, d].T

        # [s, g, k, q, n] = hr[16g+4q+k, n, s]
        hr5 = hr.reshape(8, 4, 4, M, S)  # [g, q, k, n, s]
        ht = np.zeros((32, 8, 4, 4, M), np.float16)
        ht[:S] = hr5.transpose(4, 0, 2, 1, 3)

        in_maps.append({"hn_in": hn, "b_in": bsh, "ht_in": ht})
    return in_maps


def finalize(
    results: list[dict[str, np.ndarray]],
    H: np.ndarray,
    noise_eps: np.ndarray,
) -> np.ndarray:
    nn32 = np.arange(C4)
    ll = np.arange(L)
    X0 = np.empty((B, M, M, L), np.float32)
    sumsq = 0.0
    for core in range(N_CORES):
        b, half = core // 2, core % 2
        m0 = half * ROWS
        yt = results[core]["yt_out"].astype(np.float32)  # [S, 8, 4, 4, W]
        rect = results[core]["rect_out"].astype(np.float32)  # [128,16,2,8,55]

        sumsq += float(np.sum(yt.astype(np.float64) ** 2))

        # X0[p, n, l]: p = 8*w4 + rw; rect[32*(c4%4)+n', w4, c4//4, rw, n'+l]
        x0c = np.empty((ROWS, M, L), np.float32)
        for c4 in range(NCH4):
            cb, bk = c4 % 4, c4 // 4
            blk = rect[32 * cb : 32 * cb + C4, :, bk, :, :]  # [32, 16, 8, 55]
            blk = blk.transpose(1, 2, 0, 3).reshape(ROWS, C4, WIN4)
            x0c[:, 32 * c4 : 32 * c4 + C4, :] = blk[
                :, nn32[:, None], nn32[:, None] + ll[None, :]
            ]
        X0[b, m0 : m0 + ROWS] = x0c
    sigm = sumsq / (M * W * B * 10.0 ** (NOISE_DB / 10.0))

    H2 = H[0, :, :, 0, :]
    hsum = H2.sum(axis=-1)  # (M, M)
    nwin = np.lib.stride_tricks.sliding_window_view(noise_eps[:, :, :, 0], L, axis=2)
    X = X0 + np.float32(np.sqrt(sigm)) * (hsum[None, :, :, None] * nwin)
    X = X.astype(np.float32, copy=False)
    return X / X.max()


_NC_CACHE: bass.Bass | None = None


def kernel(x: np.ndarray, H: np.ndarray, noise_eps: np.ndarray) -> np.ndarray:
    global _NC_CACHE
    x = np.asarray(x, dtype=np.float32)
    H = np.asarray(H, dtype=np.float32)
    noise_eps = np.asarray(noise_eps, dtype=np.float32)
    if _NC_CACHE is None:
        _NC_CACHE = build_bass()
    in_maps = shard_inputs(x, H)
    res = run_bass_kernel_spmd(_NC_CACHE, in_maps, core_ids=list(range(N_CORES)))
    return finalize(res.results, H, noise_eps)
